# revision 9
# baseline (speedup 1.0000x reference)
"""AttentionBlock (GroupNorm + single-head self-attention + proj + residual) on 8 Trainium2
NeuronCores, data-parallel over the batch (16 samples -> 2 per core).

Per-sample math (C=512 channels, N=1024 tokens = 32x32 spatial):
  h   = GroupNorm(x; 8 groups) * w + b           [C, N]
  q,k = Wq@h + bq, Wk@h + bk                     [C, N]   (c on partitions)
  vT  = h^T @ WvT (+bv deferred past softmax)    [N, C]   (tokens on partitions)
  S^T = k^T q * scale                            [j, i]   (j on partitions)
  E   = exp(S^T)       den[i] = sum_j E[j,i]     (den via ones-matmuls, i on partitions)
  oT[i,c] = sum_j E[j,i] vT[j,c];  oT *= 1/den[i]
  out = transpose(oT) + bv                       [C, N]
  y   = (x + bp) + Wp@out                        [C, N]

All matmul operands are bf16 (PSUM accumulates fp32); softmax/statistics are fp32.
Softmax skips max-subtraction: scores are ~N(0,1) (|s| < ~7), exp() is safe in fp32.
"""

import numpy as np
import ml_dtypes

import concourse.bacc as bacc
import concourse.tile as tile
from concourse import mybir
from concourse.bass_utils import run_bass_kernel_spmd
from concourse.hw_specs import get_activation_tables as _gat
from concourse.masks import make_identity

F32 = mybir.dt.float32
BF16 = mybir.dt.bfloat16
AF = mybir.ActivationFunctionType
OP = mybir.AluOpType

NCORES = 8
S = 2          # samples per core
C = 512
N = 1024       # H*W
CT = C // 128  # channel tiles
NT = N // 128  # token tiles
EPS = 1e-5
SCALE = float(C) ** -0.5

# All ACT funcs we use (Exp, Ln, Identity, Copy) live in one table set; blank out the
# other sets (keeping list positions!) so the table-load pass never alternates sets.
_ONE_SET = "natural_log_exp_and_others"


def _gat_filtered(arch):
    return {name: (fns if name == _ONE_SET else set())
            for name, fns in _gat(arch).items()}


bacc.get_activation_tables = _gat_filtered


def build_nc():
    nc = bacc.Bacc("TRN2", target_bir_lowering=False)
    x_d = nc.dram_tensor("x", [S, C, N], F32, kind="ExternalInput")
    wT_d = nc.dram_tensor("qkv_wT", [C, 3 * C], BF16, kind="ExternalInput")
    pwT_d = nc.dram_tensor("proj_wT", [C, C], BF16, kind="ExternalInput")
    nw_d = nc.dram_tensor("norm_w", [C], F32, kind="ExternalInput")
    nb_d = nc.dram_tensor("norm_b", [C], F32, kind="ExternalInput")
    qkvb_d = nc.dram_tensor("qkv_b", [3 * C], F32, kind="ExternalInput")
    pb_d = nc.dram_tensor("proj_b", [C], F32, kind="ExternalInput")
    gm_d = nc.dram_tensor("gmat", [128, 128], F32, kind="ExternalInput")
    dsc_d = nc.dram_tensor("den_scratch", [S, N], F32, kind="ExternalOutput")
    out_d = nc.dram_tensor("out", [S, C, N], F32, kind="ExternalOutput")

    with tile.TileContext(nc) as tc:
        with (
            tc.tile_pool(name="consts", bufs=1) as consts,
            tc.tile_pool(name="xp", bufs=2) as xp,
            tc.tile_pool(name="hp", bufs=2) as hp,
            tc.tile_pool(name="qp", bufs=2) as qp,
            tc.tile_pool(name="kp", bufs=2) as kp,
            tc.tile_pool(name="vp", bufs=2) as vp,
            tc.tile_pool(name="esp", bufs=1) as esp,
            tc.tile_pool(name="otp", bufs=1) as otp,
            tc.tile_pool(name="aop", bufs=1) as aop,
            tc.tile_pool(name="finp", bufs=2) as finp,
            tc.tile_pool(name="statp", bufs=4) as statp,
            tc.tile_pool(name="ps_big", bufs=2, space="PSUM") as ps_big,
            tc.tile_pool(name="ps_mid", bufs=2, space="PSUM") as ps_mid,
            tc.tile_pool(name="ps_sm", bufs=1, space="PSUM") as ps_sm,
        ):
            # ---------------- load x (issued first so its DMAs lead the queues) ----
            x_sb, h_sb, q_sb, k_sb, vT_sb = {}, {}, {}, {}, {}
            es_sb, oT_sb, ao_sb, fin_sb, recip = {}, {}, {}, {}, {}
            for s in range(S):
                x_sb[s] = xp.tile([128, CT, N], F32, tag="x", name=f"x{s}")
                for ct in range(CT):
                    nc.sync.dma_start(x_sb[s][:, ct, :], x_d[s, ct * 128:(ct + 1) * 128, :])

            # ---------------- constants ----------------
            wT = consts.tile([128, CT, 3 * C], BF16, tag="wT")
            wT_r = wT_d.ap().rearrange("(kc p) o -> p kc o", p=128)
            for kc in range(CT):
                nc.sync.dma_start(wT[:, kc, :], wT_r[:, kc, :])
            pwT = consts.tile([128, CT, C], BF16, tag="pwT")
            pwT_r = pwT_d.ap().rearrange("(kc p) o -> p kc o", p=128)
            for kc in range(CT):
                nc.sync.dma_start(pwT[:, kc, :], pwT_r[:, kc, :])
            gmat = consts.tile([128, 128], F32, tag="gmat")
            nc.sync.dma_start(gmat, gm_d.ap())
            ident = consts.tile([128, 128], F32, tag="ident")
            make_identity(nc, ident)
            ones_bf = consts.tile([128, 1], BF16, tag="ones")
            nc.vector.memset(ones_bf, 1.0)
            epsb = consts.tile([128, 1], F32, tag="eps")
            nc.vector.memset(epsb, EPS)
            nw = consts.tile([128, CT], F32, tag="nw")
            nc.sync.dma_start(nw, nw_d.ap().rearrange("(t p) -> p t", p=128))
            nb = consts.tile([128, CT], F32, tag="nb")
            nc.sync.dma_start(nb, nb_d.ap().rearrange("(t p) -> p t", p=128))
            qb = consts.tile([128, CT], F32, tag="qb")
            nc.sync.dma_start(qb, qkvb_d.ap()[0:C].rearrange("(t p) -> p t", p=128))
            kb = consts.tile([128, CT], F32, tag="kb")
            nc.sync.dma_start(kb, qkvb_d.ap()[C:2 * C].rearrange("(t p) -> p t", p=128))
            vb = consts.tile([128, CT], F32, tag="vb")
            nc.sync.dma_start(vb, qkvb_d.ap()[2 * C:3 * C].rearrange("(t p) -> p t", p=128))
            pb = consts.tile([128, CT], F32, tag="pb")
            nc.sync.dma_start(pb, pb_d.ap().rearrange("(t p) -> p t", p=128))

            # warm the ACT table set under the DMA shadow
            warm = statp.tile([128, 1], F32, tag="tmp", name="warm")
            nc.scalar.activation(warm, epsb, AF.Exp, bias=0.0, scale=1.0)


            for s in range(S):
                h_sb[s] = hp.tile([128, CT, N], BF16, tag="h", name=f"h{s}")
                # per-partition (mean, E[x^2]) for all 4 c-tiles: mv[:, ct, 0:2]
                mv = statp.tile([128, CT, 2], F32, tag="mv", name=f"mv{s}")
                for ct in range(CT):
                    st = statp.tile([128, 2, 6], F32, tag="bnst")
                    for i in range(2):
                        nc.vector.bn_stats(st[:, i, :], x_sb[s][:, ct, i * 512:(i + 1) * 512])
                    nc.vector.bn_aggr(mv[:, ct, :], st)
                # E[x^2] = var + mean^2 (batched over all c-tiles, strided [128,4] views)
                msq = statp.tile([128, CT, 2], F32, tag="msq", name=f"msq{s}")
                nc.vector.tensor_copy(msq[:, :, 0], mv[:, :, 0])
                nc.vector.tensor_tensor(msq[:, :, 1], mv[:, :, 0], mv[:, :, 0], OP.mult)
                nc.vector.tensor_tensor(msq[:, :, 1], msq[:, :, 1], mv[:, :, 1], OP.add)
                # group-average + broadcast back to all partitions: ONE matmul, all c-tiles
                gps = ps_mid.tile([128, 512], F32, tag="mid", name=f"gps{s}")
                nc.tensor.matmul(gps[:, 0:2 * CT], lhsT=gmat,
                                 rhs=msq.rearrange("p a b -> p (a b)"),
                                 start=True, stop=True)
                gst = statp.tile([128, CT, 2], F32, tag="gst", name=f"gst{s}")
                nc.vector.tensor_copy(gst.rearrange("p a b -> p (a b)"), gps[:, 0:2 * CT])
                # scale = rstd * w ; shift = b - mean * scale   (all c-tiles at once)
                sc = statp.tile([128, CT, 2], F32, tag="sc", name=f"sc{s}")
                tmp = statp.tile([128, CT], F32, tag="tmp", name=f"tmp{s}")
                nc.vector.tensor_tensor(tmp, gst[:, :, 0], gst[:, :, 0], OP.mult)
                nc.vector.tensor_tensor(tmp, gst[:, :, 1], tmp, OP.subtract)  # var
                # rstd = exp(-0.5*ln(var+eps)); Ln+Exp live in one ACT table set
                nc.scalar.activation(tmp, tmp, AF.Ln, bias=epsb, scale=1.0)
                nc.scalar.activation(tmp, tmp, AF.Exp, bias=0.0, scale=-0.5)
                nc.vector.tensor_tensor(sc[:, :, 0], tmp, nw, OP.mult)
                nc.vector.tensor_tensor(tmp, gst[:, :, 0], sc[:, :, 0], OP.mult)
                nc.vector.tensor_tensor(sc[:, :, 1], nb, tmp, OP.subtract)
                for ct in range(CT):
                    nc.scalar.activation(h_sb[s][:, ct, :], x_sb[s][:, ct, :], AF.Identity,
                                         bias=sc[:, ct, 1:2], scale=sc[:, ct, 0:1])
                    # x is no longer needed raw; pre-add proj bias for the residual
                    nc.vector.tensor_scalar(x_sb[s][:, ct, :], x_sb[s][:, ct, :],
                                            pb[:, ct:ct + 1], None, OP.add)

            # ---------------- QKV ----------------
            for s in range(S):
                q_sb[s] = qp.tile([128, CT, N], BF16, tag="q", name=f"q{s}")
                k_sb[s] = kp.tile([128, CT, N], BF16, tag="k", name=f"k{s}")
                vT_sb[s] = vp.tile([128, NT, C], BF16, tag="vT", name=f"vT{s}")
                for mo in range(CT):
                    for nch in range(2):
                        ps = ps_mid.tile([128, 512], F32, tag="mid")
                        for kc in range(CT):
                            nc.tensor.matmul(ps, lhsT=wT[:, kc, mo * 128:(mo + 1) * 128],
                                             rhs=h_sb[s][:, kc, nch * 512:(nch + 1) * 512],
                                             start=(kc == 0), stop=(kc == CT - 1))
                        nc.vector.tensor_scalar(q_sb[s][:, mo, nch * 512:(nch + 1) * 512],
                                                ps, qb[:, mo:mo + 1], None, OP.add)
                for mo in range(CT):
                    for nch in range(2):
                        ps = ps_mid.tile([128, 512], F32, tag="mid")
                        for kc in range(CT):
                            nc.tensor.matmul(ps, lhsT=wT[:, kc, C + mo * 128:C + (mo + 1) * 128],
                                             rhs=h_sb[s][:, kc, nch * 512:(nch + 1) * 512],
                                             start=(kc == 0), stop=(kc == CT - 1))
                        nc.scalar.activation(k_sb[s][:, mo, nch * 512:(nch + 1) * 512], ps,
                                             AF.Identity, bias=kb[:, mo:mo + 1], scale=1.0)
                for it in range(NT):
                    ps = ps_mid.tile([128, 512], F32, tag="mid")
                    for kc in range(CT):
                        nc.tensor.matmul(ps, lhsT=h_sb[s][:, kc, it * 128:(it + 1) * 128],
                                         rhs=wT[:, kc, 2 * C:3 * C],
                                         start=(kc == 0), stop=(kc == CT - 1))
                    nc.vector.tensor_copy(vT_sb[s][:, it, :], ps)

            # ---------------- S^T, exp, den ----------------
            for s in range(S):
                es_sb[s] = esp.tile([128, NT, N], BF16, tag="es", name=f"es{s}")
                den_ps = ps_sm.tile([1, N], F32, tag="den", name=f"den{s}")
                for jt in range(NT):
                    ps = ps_big.tile([128, N], F32, tag="big")
                    for kc in range(CT):
                        for nch in range(2):
                            nc.tensor.matmul(ps[:, nch * 512:(nch + 1) * 512],
                                             lhsT=k_sb[s][:, kc, jt * 128:(jt + 1) * 128],
                                             rhs=q_sb[s][:, kc, nch * 512:(nch + 1) * 512],
                                             start=(kc == 0), stop=(kc == CT - 1))
                    nc.scalar.activation(es_sb[s][:, jt, :], ps, AF.Exp, bias=0.0, scale=SCALE)
                    # den row += column-sum of this es tile (each 512-chunk is its own
                    # psum bank, so per-chunk start/stop groups are safe)
                    for nch in range(2):
                        nc.tensor.matmul(den_ps[0:1, nch * 512:(nch + 1) * 512],
                                         lhsT=ones_bf,
                                         rhs=es_sb[s][:, jt, nch * 512:(nch + 1) * 512],
                                         start=(jt == 0), stop=(jt == NT - 1))
                den_row = statp.tile([1, N], F32, tag="denrow", name=f"denrow{s}")
                nc.vector.reciprocal(den_row, den_ps)
                # scatter the 1x1024 row to [128, 8] (i on partitions) via a DRAM bounce
                # (SBUF APs cannot move free elements onto partitions directly)
                nc.gpsimd.dma_start(dsc_d[s:s + 1, :], den_row)
                recip[s] = statp.tile([128, NT], F32, tag="recip", name=f"recip{s}")
                nc.gpsimd.dma_start(recip[s], dsc_d[s].rearrange("(t p) -> p t", p=128))

            # ---------------- AV (-> oT[i, c]) ----------------
            for s in range(S):
                oT_sb[s] = otp.tile([128, NT, C], F32, tag="oT", name=f"oT{s}")
                for it in range(NT):
                    ps = ps_mid.tile([128, 512], F32, tag="mid")
                    for jt in range(NT):
                        nc.tensor.matmul(ps, lhsT=es_sb[s][:, jt, it * 128:(it + 1) * 128],
                                         rhs=vT_sb[s][:, jt, :],
                                         start=(jt == 0), stop=(jt == NT - 1))
                    nc.vector.tensor_scalar(oT_sb[s][:, it, :], ps, recip[s][:, it:it + 1],
                                            None, OP.mult)

            # ---------------- transpose oT -> out[c, n] (+bv) ----------------
            for s in range(S):
                ao_sb[s] = aop.tile([128, CT, N], BF16, tag="ao", name=f"ao{s}")
                for ct in range(CT):
                    ps = ps_big.tile([128, N], F32, tag="big")
                    for it in range(NT):
                        nc.tensor.transpose(ps[:, it * 128:(it + 1) * 128],
                                            oT_sb[s][:, it, ct * 128:(ct + 1) * 128], ident)
                    nc.scalar.activation(ao_sb[s][:, ct, :], ps, AF.Identity,
                                         bias=vb[:, ct:ct + 1], scale=1.0)

            # ---------------- proj + residual ----------------
            for s in range(S):
                fin_sb[s] = finp.tile([128, CT, N], F32, tag="fin", name=f"fin{s}")
                for mo in range(CT):
                    for nch in range(2):
                        ps = ps_mid.tile([128, 512], F32, tag="mid")
                        for kc in range(CT):
                            nc.tensor.matmul(ps, lhsT=pwT[:, kc, mo * 128:(mo + 1) * 128],
                                             rhs=ao_sb[s][:, kc, nch * 512:(nch + 1) * 512],
                                             start=(kc == 0), stop=(kc == CT - 1))
                        nc.vector.tensor_tensor(fin_sb[s][:, mo, nch * 512:(nch + 1) * 512],
                                                ps, x_sb[s][:, mo, nch * 512:(nch + 1) * 512],
                                                OP.add)
                for ct in range(CT):
                    nc.sync.dma_start(out_d[s, ct * 128:(ct + 1) * 128, :], fin_sb[s][:, ct, :])

    nc.finalize()
    return nc


_NC_CACHE = None
LAST_EXEC_NS = None
LAST_RESULTS = None


def _get_nc():
    global _NC_CACHE
    if _NC_CACHE is None:
        _NC_CACHE = build_nc()
    return _NC_CACHE


def make_gmat():
    g = np.zeros((128, 128), np.float32)
    g[:64, :64] = 1.0 / 64
    g[64:, 64:] = 1.0 / 64
    return g


def make_in_maps(x, norm_w, norm_b, qkv_w, qkv_b, proj_w, proj_b):
    bf = ml_dtypes.bfloat16
    x = np.asarray(x, np.float32)
    B = x.shape[0]
    x_r = np.ascontiguousarray(x.reshape(B, C, N))
    qkv_wT = np.ascontiguousarray(np.asarray(qkv_w, np.float32).T).astype(bf)
    proj_wT = np.ascontiguousarray(np.asarray(proj_w, np.float32).T).astype(bf)
    common = {
        "qkv_wT": qkv_wT,
        "proj_wT": proj_wT,
        "norm_w": np.ascontiguousarray(np.asarray(norm_w, np.float32)),
        "norm_b": np.ascontiguousarray(np.asarray(norm_b, np.float32)),
        "qkv_b": np.ascontiguousarray(np.asarray(qkv_b, np.float32)),
        "proj_b": np.ascontiguousarray(np.asarray(proj_b, np.float32)),
        "gmat": make_gmat(),
    }
    per = B // NCORES
    return [dict(common, x=np.ascontiguousarray(x_r[c * per:(c + 1) * per]))
            for c in range(NCORES)]


def kernel(x, norm_w, norm_b, qkv_w, qkv_b, proj_w, proj_b, _trace=False):
    global LAST_EXEC_NS, LAST_RESULTS
    x = np.asarray(x)
    B, C_, H, W = x.shape
    in_maps = make_in_maps(x, norm_w, norm_b, qkv_w, qkv_b, proj_w, proj_b)
    res = run_bass_kernel_spmd(_get_nc(), in_maps, core_ids=list(range(NCORES)),
                               trace=_trace)
    LAST_EXEC_NS = res.exec_time_ns
    LAST_RESULTS = res
    out = np.concatenate([res.results[c]["out"] for c in range(NCORES)], axis=0)
    return out.reshape(B, C_, H, W).astype(np.float32)


# revision 10
# speedup vs baseline: 1.1118x; 1.1118x over previous
"""AttentionBlock (GroupNorm + single-head self-attention + proj + residual) on 8 Trainium2
NeuronCores, data-parallel over the batch (16 samples -> 2 per core).

Per-sample math (C=512 channels, N=1024 tokens = 32x32 spatial):
  h   = GroupNorm(x; 8 groups) * w + b           [C, N]
  q,k = Wq@h + bq, Wk@h + bk                     [C, N]   (c on partitions)
  vT  = h^T @ WvT (+bv deferred past softmax)    [N, C]   (tokens on partitions)
  S^T = k^T q * scale                            [j, i]   (j on partitions)
  E   = exp(S^T)       den[i] = sum_j E[j,i]     (den via ones-matmuls, i on partitions)
  oT[i,c] = sum_j E[j,i] vT[j,c];  oT *= 1/den[i]
  out = transpose(oT) + bv                       [C, N]
  y   = (x + bp) + Wp@out                        [C, N]

All matmul operands are bf16 (PSUM accumulates fp32); softmax/statistics are fp32.
Softmax skips max-subtraction: scores are ~N(0,1) (|s| < ~7), exp() is safe in fp32.
"""

import numpy as np
import ml_dtypes

import concourse.bacc as bacc
import concourse.tile as tile
from concourse import mybir
from concourse.bass_utils import run_bass_kernel_spmd
from concourse.hw_specs import get_activation_tables as _gat
from concourse.masks import make_identity

F32 = mybir.dt.float32
BF16 = mybir.dt.bfloat16
AF = mybir.ActivationFunctionType
OP = mybir.AluOpType

NCORES = 8
S = 2          # samples per core
C = 512
N = 1024       # H*W
CT = C // 128  # channel tiles
NT = N // 128  # token tiles
EPS = 1e-5
SCALE = float(C) ** -0.5

# All ACT funcs we use (Exp, Ln, Identity, Copy) live in one table set; blank out the
# other sets (keeping list positions!) so the table-load pass never alternates sets.
_ONE_SET = "natural_log_exp_and_others"


def _gat_filtered(arch):
    return {name: (fns if name == _ONE_SET else set())
            for name, fns in _gat(arch).items()}


bacc.get_activation_tables = _gat_filtered


def build_nc():
    nc = bacc.Bacc("TRN2", target_bir_lowering=False)
    x_d = nc.dram_tensor("x", [S, C, N], F32, kind="ExternalInput")
    wT_d = nc.dram_tensor("qkv_wT", [C, 3 * C], BF16, kind="ExternalInput")
    pwT_d = nc.dram_tensor("proj_wT", [C, C], BF16, kind="ExternalInput")
    nw_d = nc.dram_tensor("norm_w", [C], F32, kind="ExternalInput")
    nb_d = nc.dram_tensor("norm_b", [C], F32, kind="ExternalInput")
    qkvb_d = nc.dram_tensor("qkv_b", [3 * C], F32, kind="ExternalInput")
    pb_d = nc.dram_tensor("proj_b", [C], F32, kind="ExternalInput")
    gm_d = nc.dram_tensor("gmat", [128, 128], F32, kind="ExternalInput")
    out_d = nc.dram_tensor("out", [S, C, N], F32, kind="ExternalOutput")

    with tile.TileContext(nc) as tc:
        with (
            tc.tile_pool(name="consts", bufs=1) as consts,
            tc.tile_pool(name="xp", bufs=2) as xp,
            tc.tile_pool(name="hp", bufs=2) as hp,
            tc.tile_pool(name="qp", bufs=2) as qp,
            tc.tile_pool(name="kp", bufs=2) as kp,
            tc.tile_pool(name="vp", bufs=2) as vp,
            tc.tile_pool(name="esp", bufs=1) as esp,
            tc.tile_pool(name="otp", bufs=1) as otp,
            tc.tile_pool(name="aop", bufs=1) as aop,
            tc.tile_pool(name="finp", bufs=2) as finp,
            tc.tile_pool(name="statp", bufs=4) as statp,
            tc.tile_pool(name="ps_big", bufs=2, space="PSUM") as ps_big,
            tc.tile_pool(name="ps_mid", bufs=2, space="PSUM") as ps_mid,
            tc.tile_pool(name="ps_sm", bufs=1, space="PSUM") as ps_sm,
        ):
            # ---------------- load x(s0) and weights first, then x(s1) ----------
            x_sb, h_sb, q_sb, k_sb, vT_sb = {}, {}, {}, {}, {}
            es_sb, oT_sb, ao_sb, fin_sb, recip = {}, {}, {}, {}, {}
            wT = consts.tile([128, CT, 3 * C], BF16, tag="wT")
            wT_r = wT_d.ap().rearrange("(kc p) o -> p kc o", p=128)
            for s in range(S):
                x_sb[s] = xp.tile([128, CT, N], F32, tag="x", name=f"x{s}")
            for ct in range(CT):
                nc.sync.dma_start(x_sb[0][:, ct, :], x_d[0, ct * 128:(ct + 1) * 128, :])
            for kc in range(CT):
                nc.sync.dma_start(wT[:, kc, :], wT_r[:, kc, :])
            for ct in range(CT):
                nc.sync.dma_start(x_sb[1][:, ct, :], x_d[1, ct * 128:(ct + 1) * 128, :])

            # ---------------- constants ----------------
            pwT = consts.tile([128, CT, C], BF16, tag="pwT")
            pwT_r = pwT_d.ap().rearrange("(kc p) o -> p kc o", p=128)
            for kc in range(CT):
                nc.sync.dma_start(pwT[:, kc, :], pwT_r[:, kc, :])
            gmat = consts.tile([128, 128], F32, tag="gmat")
            nc.sync.dma_start(gmat, gm_d.ap())
            ident = consts.tile([128, 128], F32, tag="ident")
            make_identity(nc, ident)
            ones_bf = consts.tile([128, 1], BF16, tag="ones")
            nc.vector.memset(ones_bf, 1.0)
            epsb = consts.tile([128, 1], F32, tag="eps")
            nc.vector.memset(epsb, EPS)
            nw = consts.tile([128, CT], F32, tag="nw")
            nc.sync.dma_start(nw, nw_d.ap().rearrange("(t p) -> p t", p=128))
            nb = consts.tile([128, CT], F32, tag="nb")
            nc.sync.dma_start(nb, nb_d.ap().rearrange("(t p) -> p t", p=128))
            qb = consts.tile([128, CT], F32, tag="qb")
            nc.sync.dma_start(qb, qkvb_d.ap()[0:C].rearrange("(t p) -> p t", p=128))
            kb = consts.tile([128, CT], F32, tag="kb")
            nc.sync.dma_start(kb, qkvb_d.ap()[C:2 * C].rearrange("(t p) -> p t", p=128))
            vb = consts.tile([128, CT], F32, tag="vb")
            nc.sync.dma_start(vb, qkvb_d.ap()[2 * C:3 * C].rearrange("(t p) -> p t", p=128))
            pb = consts.tile([128, CT], F32, tag="pb")
            nc.sync.dma_start(pb, pb_d.ap().rearrange("(t p) -> p t", p=128))

            # warm the ACT table set under the DMA shadow
            warm = statp.tile([128, 1], F32, tag="tmp", name="warm")
            nc.scalar.activation(warm, epsb, AF.Exp, bias=0.0, scale=1.0)


            for s in range(S):
                h_sb[s] = hp.tile([128, CT, N], BF16, tag="h", name=f"h{s}")
                # per-partition (mean, E[x^2]) for all 4 c-tiles: mv[:, ct, 0:2]
                mv = statp.tile([128, CT, 2], F32, tag="mv", name=f"mv{s}")
                for ct in range(CT):
                    st = statp.tile([128, 2, 6], F32, tag="bnst")
                    for i in range(2):
                        nc.vector.bn_stats(st[:, i, :], x_sb[s][:, ct, i * 512:(i + 1) * 512])
                    nc.vector.bn_aggr(mv[:, ct, :], st)
                # E[x^2] = var + mean^2 (batched over all c-tiles, strided [128,4] views)
                msq = statp.tile([128, CT, 2], F32, tag="msq", name=f"msq{s}")
                nc.vector.tensor_copy(msq[:, :, 0], mv[:, :, 0])
                nc.vector.tensor_tensor(msq[:, :, 1], mv[:, :, 0], mv[:, :, 0], OP.mult)
                nc.vector.tensor_tensor(msq[:, :, 1], msq[:, :, 1], mv[:, :, 1], OP.add)
                # group-average + broadcast back to all partitions: ONE matmul, all c-tiles
                gps = ps_mid.tile([128, 512], F32, tag="mid", name=f"gps{s}")
                nc.tensor.matmul(gps[:, 0:2 * CT], lhsT=gmat,
                                 rhs=msq.rearrange("p a b -> p (a b)"),
                                 start=True, stop=True)
                gst = statp.tile([128, CT, 2], F32, tag="gst", name=f"gst{s}")
                nc.vector.tensor_copy(gst.rearrange("p a b -> p (a b)"), gps[:, 0:2 * CT])
                # scale = rstd * w ; shift = b - mean * scale   (all c-tiles at once)
                sc = statp.tile([128, CT, 2], F32, tag="sc", name=f"sc{s}")
                tmp = statp.tile([128, CT], F32, tag="tmp", name=f"tmp{s}")
                nc.vector.tensor_tensor(tmp, gst[:, :, 0], gst[:, :, 0], OP.mult)
                nc.vector.tensor_tensor(tmp, gst[:, :, 1], tmp, OP.subtract)  # var
                # rstd = exp(-0.5*ln(var+eps)); Ln+Exp live in one ACT table set
                nc.scalar.activation(tmp, tmp, AF.Ln, bias=epsb, scale=1.0)
                nc.scalar.activation(tmp, tmp, AF.Exp, bias=0.0, scale=-0.5)
                nc.vector.tensor_tensor(sc[:, :, 0], tmp, nw, OP.mult)
                nc.vector.tensor_tensor(tmp, gst[:, :, 0], sc[:, :, 0], OP.mult)
                nc.vector.tensor_tensor(sc[:, :, 1], nb, tmp, OP.subtract)
                for ct in range(CT):
                    nc.scalar.activation(h_sb[s][:, ct, :], x_sb[s][:, ct, :], AF.Identity,
                                         bias=sc[:, ct, 1:2], scale=sc[:, ct, 0:1])
                    # x is no longer needed raw; pre-add proj bias for the residual
                    nc.vector.tensor_scalar(x_sb[s][:, ct, :], x_sb[s][:, ct, :],
                                            pb[:, ct:ct + 1], None, OP.add)

            # ---------------- QKV ----------------
            for s in range(S):
                q_sb[s] = qp.tile([128, CT, N], BF16, tag="q", name=f"q{s}")
                k_sb[s] = kp.tile([128, CT, N], BF16, tag="k", name=f"k{s}")
                vT_sb[s] = vp.tile([128, NT, C], BF16, tag="vT", name=f"vT{s}")
                for mo in range(CT):
                    for nch in range(2):
                        ps = ps_mid.tile([128, 512], F32, tag="mid")
                        for kc in range(CT):
                            nc.tensor.matmul(ps, lhsT=wT[:, kc, mo * 128:(mo + 1) * 128],
                                             rhs=h_sb[s][:, kc, nch * 512:(nch + 1) * 512],
                                             start=(kc == 0), stop=(kc == CT - 1))
                        nc.vector.tensor_scalar(q_sb[s][:, mo, nch * 512:(nch + 1) * 512],
                                                ps, qb[:, mo:mo + 1], None, OP.add)
                for mo in range(CT):
                    for nch in range(2):
                        ps = ps_mid.tile([128, 512], F32, tag="mid")
                        for kc in range(CT):
                            nc.tensor.matmul(ps, lhsT=wT[:, kc, C + mo * 128:C + (mo + 1) * 128],
                                             rhs=h_sb[s][:, kc, nch * 512:(nch + 1) * 512],
                                             start=(kc == 0), stop=(kc == CT - 1))
                        nc.scalar.activation(k_sb[s][:, mo, nch * 512:(nch + 1) * 512], ps,
                                             AF.Identity, bias=kb[:, mo:mo + 1], scale=1.0)
                for it in range(NT):
                    ps = ps_mid.tile([128, 512], F32, tag="mid")
                    for kc in range(CT):
                        nc.tensor.matmul(ps, lhsT=h_sb[s][:, kc, it * 128:(it + 1) * 128],
                                         rhs=wT[:, kc, 2 * C:3 * C],
                                         start=(kc == 0), stop=(kc == CT - 1))
                    nc.vector.tensor_copy(vT_sb[s][:, it, :], ps)

            # ---------------- S^T, exp, den ----------------
            for s in range(S):
                es_sb[s] = esp.tile([128, NT, N], BF16, tag="es", name=f"es{s}")
                den_ps = ps_sm.tile([128, NT], F32, tag="den", name=f"den{s}")
                nc.vector.memset(den_ps, 0.0)
                for jt in range(NT):
                    ps = ps_big.tile([128, N], F32, tag="big")
                    for kc in range(CT):
                        for nch in range(2):
                            nc.tensor.matmul(ps[:, nch * 512:(nch + 1) * 512],
                                             lhsT=k_sb[s][:, kc, jt * 128:(jt + 1) * 128],
                                             rhs=q_sb[s][:, kc, nch * 512:(nch + 1) * 512],
                                             start=(kc == 0), stop=(kc == CT - 1))
                    nc.scalar.activation(es_sb[s][:, jt, :], ps, AF.Exp, bias=0.0, scale=SCALE)
                    # den[i] += sum_j(this tile), one tiny matmul per i-chunk; accumulate
                    # into a memset psum bank (start=False: first write per element
                    # overwrites or adds to zeroed data -- correct either way, and these
                    # small matmuls fill PE bubbles during the S^T phase)
                    for ic in range(NT):
                        nc.tensor.matmul(den_ps[:, ic:ic + 1],
                                         lhsT=es_sb[s][:, jt, ic * 128:(ic + 1) * 128],
                                         rhs=ones_bf,
                                         start=False, stop=False, skip_group_check=True)
                recip[s] = statp.tile([128, NT], F32, tag="recip", name=f"recip{s}")
                nc.vector.reciprocal(recip[s], den_ps)

            # ---------------- AV (-> oT[i, c]) ----------------
            for s in range(S):
                oT_sb[s] = otp.tile([128, NT, C], F32, tag="oT", name=f"oT{s}")
                for it in range(NT):
                    ps = ps_mid.tile([128, 512], F32, tag="mid")
                    for jt in range(NT):
                        nc.tensor.matmul(ps, lhsT=es_sb[s][:, jt, it * 128:(it + 1) * 128],
                                         rhs=vT_sb[s][:, jt, :],
                                         start=(jt == 0), stop=(jt == NT - 1))
                    nc.vector.tensor_scalar(oT_sb[s][:, it, :], ps, recip[s][:, it:it + 1],
                                            None, OP.mult)

            # ---------------- transpose oT -> out[c, n] (+bv) ----------------
            for s in range(S):
                ao_sb[s] = aop.tile([128, CT, N], BF16, tag="ao", name=f"ao{s}")
                for ct in range(CT):
                    ps = ps_big.tile([128, N], F32, tag="big")
                    for it in range(NT):
                        nc.tensor.transpose(ps[:, it * 128:(it + 1) * 128],
                                            oT_sb[s][:, it, ct * 128:(ct + 1) * 128], ident)
                    nc.scalar.activation(ao_sb[s][:, ct, :], ps, AF.Identity,
                                         bias=vb[:, ct:ct + 1], scale=1.0)

            # ---------------- proj + residual ----------------
            for s in range(S):
                fin_sb[s] = finp.tile([128, CT, N], F32, tag="fin", name=f"fin{s}")
                for mo in range(CT):
                    for nch in range(2):
                        ps = ps_mid.tile([128, 512], F32, tag="mid")
                        for kc in range(CT):
                            nc.tensor.matmul(ps, lhsT=pwT[:, kc, mo * 128:(mo + 1) * 128],
                                             rhs=ao_sb[s][:, kc, nch * 512:(nch + 1) * 512],
                                             start=(kc == 0), stop=(kc == CT - 1))
                        nc.vector.tensor_tensor(fin_sb[s][:, mo, nch * 512:(nch + 1) * 512],
                                                ps, x_sb[s][:, mo, nch * 512:(nch + 1) * 512],
                                                OP.add)
                for ct in range(CT):
                    nc.sync.dma_start(out_d[s, ct * 128:(ct + 1) * 128, :], fin_sb[s][:, ct, :])

    nc.finalize()
    return nc


_NC_CACHE = None
LAST_EXEC_NS = None
LAST_RESULTS = None


def _get_nc():
    global _NC_CACHE
    if _NC_CACHE is None:
        _NC_CACHE = build_nc()
    return _NC_CACHE


def make_gmat():
    g = np.zeros((128, 128), np.float32)
    g[:64, :64] = 1.0 / 64
    g[64:, 64:] = 1.0 / 64
    return g


def make_in_maps(x, norm_w, norm_b, qkv_w, qkv_b, proj_w, proj_b):
    bf = ml_dtypes.bfloat16
    x = np.asarray(x, np.float32)
    B = x.shape[0]
    x_r = np.ascontiguousarray(x.reshape(B, C, N))
    qkv_wT = np.ascontiguousarray(np.asarray(qkv_w, np.float32).T).astype(bf)
    proj_wT = np.ascontiguousarray(np.asarray(proj_w, np.float32).T).astype(bf)
    common = {
        "qkv_wT": qkv_wT,
        "proj_wT": proj_wT,
        "norm_w": np.ascontiguousarray(np.asarray(norm_w, np.float32)),
        "norm_b": np.ascontiguousarray(np.asarray(norm_b, np.float32)),
        "qkv_b": np.ascontiguousarray(np.asarray(qkv_b, np.float32)),
        "proj_b": np.ascontiguousarray(np.asarray(proj_b, np.float32)),
        "gmat": make_gmat(),
    }
    per = B // NCORES
    return [dict(common, x=np.ascontiguousarray(x_r[c * per:(c + 1) * per]))
            for c in range(NCORES)]


def kernel(x, norm_w, norm_b, qkv_w, qkv_b, proj_w, proj_b, _trace=False):
    global LAST_EXEC_NS, LAST_RESULTS
    x = np.asarray(x)
    B, C_, H, W = x.shape
    in_maps = make_in_maps(x, norm_w, norm_b, qkv_w, qkv_b, proj_w, proj_b)
    res = run_bass_kernel_spmd(_get_nc(), in_maps, core_ids=list(range(NCORES)),
                               trace=_trace)
    LAST_EXEC_NS = res.exec_time_ns
    LAST_RESULTS = res
    out = np.concatenate([res.results[c]["out"] for c in range(NCORES)], axis=0)
    return out.reshape(B, C_, H, W).astype(np.float32)


# revision 11
# speedup vs baseline: 1.1398x; 1.0252x over previous
"""AttentionBlock (GroupNorm + single-head self-attention + proj + residual) on 8 Trainium2
NeuronCores, data-parallel over the batch (16 samples -> 2 per core).

Per-sample math (C=512 channels, N=1024 tokens = 32x32 spatial):
  h   = GroupNorm(x; 8 groups) * w + b           [C, N]
  q,k = Wq@h + bq, Wk@h + bk                     [C, N]   (c on partitions)
  vT  = h^T @ WvT (+bv deferred past softmax)    [N, C]   (tokens on partitions)
  S^T = k^T q * scale                            [j, i]   (j on partitions)
  E   = exp(S^T)       den[i] = sum_j E[j,i]     (den via ones-matmuls, i on partitions)
  oT[i,c] = sum_j E[j,i] vT[j,c];  oT *= 1/den[i]
  out = transpose(oT) + bv                       [C, N]
  y   = (x + bp) + Wp@out                        [C, N]

All matmul operands are bf16 (PSUM accumulates fp32); softmax/statistics are fp32.
Softmax skips max-subtraction: scores are ~N(0,1) (|s| < ~7), exp() is safe in fp32.
"""

import numpy as np
import ml_dtypes

import concourse.bacc as bacc
import concourse.tile as tile
from concourse import mybir
from concourse.bass_utils import run_bass_kernel_spmd
from concourse.hw_specs import get_activation_tables as _gat
from concourse.masks import make_identity

F32 = mybir.dt.float32
BF16 = mybir.dt.bfloat16
AF = mybir.ActivationFunctionType
OP = mybir.AluOpType

NCORES = 8
S = 2          # samples per core
C = 512
N = 1024       # H*W
CT = C // 128  # channel tiles
NT = N // 128  # token tiles
EPS = 1e-5
SCALE = float(C) ** -0.5

# All ACT funcs we use (Exp, Ln, Identity, Copy) live in one table set; blank out the
# other sets (keeping list positions!) so the table-load pass never alternates sets.
_ONE_SET = "natural_log_exp_and_others"


def _gat_filtered(arch):
    return {name: (fns if name == _ONE_SET else set())
            for name, fns in _gat(arch).items()}


bacc.get_activation_tables = _gat_filtered


def build_nc():
    nc = bacc.Bacc("TRN2", target_bir_lowering=False)
    x_d = nc.dram_tensor("x", [S, C, N], F32, kind="ExternalInput")
    wT_d = nc.dram_tensor("qkv_wT", [C, 3 * C], BF16, kind="ExternalInput")
    pwT_d = nc.dram_tensor("proj_wT", [C, C], BF16, kind="ExternalInput")
    nw_d = nc.dram_tensor("norm_w", [C], F32, kind="ExternalInput")
    nb_d = nc.dram_tensor("norm_b", [C], F32, kind="ExternalInput")
    qkvb_d = nc.dram_tensor("qkv_b", [3 * C], F32, kind="ExternalInput")
    pb_d = nc.dram_tensor("proj_b", [C], F32, kind="ExternalInput")
    gm_d = nc.dram_tensor("gmat", [128, 128], F32, kind="ExternalInput")
    out_d = nc.dram_tensor("out", [S, C, N], F32, kind="ExternalOutput")

    with tile.TileContext(nc) as tc:
        with (
            tc.tile_pool(name="consts", bufs=1) as consts,
            tc.tile_pool(name="xp", bufs=2) as xp,
            tc.tile_pool(name="hp", bufs=2) as hp,
            tc.tile_pool(name="qp", bufs=2) as qp,
            tc.tile_pool(name="kp", bufs=2) as kp,
            tc.tile_pool(name="vp", bufs=2) as vp,
            tc.tile_pool(name="esp", bufs=1) as esp,
            tc.tile_pool(name="otp", bufs=1) as otp,
            tc.tile_pool(name="aop", bufs=1) as aop,
            tc.tile_pool(name="finp", bufs=2) as finp,
            tc.tile_pool(name="statp", bufs=4) as statp,
            tc.tile_pool(name="ps_big", bufs=2, space="PSUM") as ps_big,
            tc.tile_pool(name="ps_mid", bufs=2, space="PSUM") as ps_mid,
            tc.tile_pool(name="ps_sm", bufs=1, space="PSUM") as ps_sm,
        ):
            # ---------------- load x(s0) and weights first, then x(s1) ----------
            x_sb, h_sb, q_sb, k_sb, vT_sb = {}, {}, {}, {}, {}
            es_sb, oT_sb, ao_sb, fin_sb, recip = {}, {}, {}, {}, {}
            wT = consts.tile([128, CT, 3 * C], BF16, tag="wT")
            wT_r = wT_d.ap().rearrange("(kc p) o -> p kc o", p=128)
            for s in range(S):
                x_sb[s] = xp.tile([128, CT, N], F32, tag="x", name=f"x{s}")
            for ct in range(CT):
                nc.sync.dma_start(x_sb[0][:, ct, :], x_d[0, ct * 128:(ct + 1) * 128, :])
            for kc in range(CT):
                nc.sync.dma_start(wT[:, kc, :], wT_r[:, kc, :])
            for ct in range(CT):
                nc.sync.dma_start(x_sb[1][:, ct, :], x_d[1, ct * 128:(ct + 1) * 128, :])

            # ---------------- constants ----------------
            pwT = consts.tile([128, CT, C], BF16, tag="pwT")
            pwT_r = pwT_d.ap().rearrange("(kc p) o -> p kc o", p=128)
            for kc in range(CT):
                nc.sync.dma_start(pwT[:, kc, :], pwT_r[:, kc, :])
            gmat = consts.tile([128, 128], F32, tag="gmat")
            nc.sync.dma_start(gmat, gm_d.ap())
            ident = consts.tile([128, 128], F32, tag="ident")
            make_identity(nc, ident)
            ones_bf = consts.tile([128, 1], BF16, tag="ones")
            nc.vector.memset(ones_bf, 1.0)
            epsb = consts.tile([128, 1], F32, tag="eps")
            nc.vector.memset(epsb, EPS)
            nw = consts.tile([128, CT], F32, tag="nw")
            nc.sync.dma_start(nw, nw_d.ap().rearrange("(t p) -> p t", p=128))
            nb = consts.tile([128, CT], F32, tag="nb")
            nc.sync.dma_start(nb, nb_d.ap().rearrange("(t p) -> p t", p=128))
            qb = consts.tile([128, CT], F32, tag="qb")
            nc.sync.dma_start(qb, qkvb_d.ap()[0:C].rearrange("(t p) -> p t", p=128))
            kb = consts.tile([128, CT], F32, tag="kb")
            nc.sync.dma_start(kb, qkvb_d.ap()[C:2 * C].rearrange("(t p) -> p t", p=128))
            vb = consts.tile([128, CT], F32, tag="vb")
            nc.sync.dma_start(vb, qkvb_d.ap()[2 * C:3 * C].rearrange("(t p) -> p t", p=128))
            pb = consts.tile([128, CT], F32, tag="pb")
            nc.sync.dma_start(pb, pb_d.ap().rearrange("(t p) -> p t", p=128))

            # warm the ACT table set under the DMA shadow
            warm = statp.tile([128, 1], F32, tag="tmp", name="warm")
            nc.scalar.activation(warm, epsb, AF.Exp, bias=0.0, scale=1.0)


            for s in range(S):
                h_sb[s] = hp.tile([128, CT, N], BF16, tag="h", name=f"h{s}")
                # per-partition (mean, E[x^2]) for all 4 c-tiles: mv[:, ct, 0:2]
                mv = statp.tile([128, CT, 2], F32, tag="mv", name=f"mv{s}")
                for ct in range(CT):
                    st = statp.tile([128, 2, 6], F32, tag="bnst")
                    for i in range(2):
                        nc.vector.bn_stats(st[:, i, :], x_sb[s][:, ct, i * 512:(i + 1) * 512])
                    nc.vector.bn_aggr(mv[:, ct, :], st)
                # E[x^2] = var + mean^2 (batched over all c-tiles, strided [128,4] views)
                msq = statp.tile([128, CT, 2], F32, tag="msq", name=f"msq{s}")
                nc.vector.tensor_copy(msq[:, :, 0], mv[:, :, 0])
                nc.vector.tensor_tensor(msq[:, :, 1], mv[:, :, 0], mv[:, :, 0], OP.mult)
                nc.vector.tensor_tensor(msq[:, :, 1], msq[:, :, 1], mv[:, :, 1], OP.add)
                # group-average + broadcast back to all partitions: ONE matmul, all c-tiles
                gps = ps_sm.tile([128, 2 * CT], F32, tag="gnagg", name=f"gps{s}")
                nc.tensor.matmul(gps[:, 0:2 * CT], lhsT=gmat,
                                 rhs=msq.rearrange("p a b -> p (a b)"),
                                 start=True, stop=True)
                gst = statp.tile([128, CT, 2], F32, tag="gst", name=f"gst{s}")
                nc.vector.tensor_copy(gst.rearrange("p a b -> p (a b)"), gps[:, 0:2 * CT])
                # scale = rstd * w ; shift = b - mean * scale   (all c-tiles at once)
                sc = statp.tile([128, CT, 2], F32, tag="sc", name=f"sc{s}")
                tmp = statp.tile([128, CT], F32, tag="tmp", name=f"tmp{s}")
                nc.vector.tensor_tensor(tmp, gst[:, :, 0], gst[:, :, 0], OP.mult)
                nc.vector.tensor_tensor(tmp, gst[:, :, 1], tmp, OP.subtract)  # var
                # rstd = exp(-0.5*ln(var+eps)); Ln+Exp live in one ACT table set
                nc.scalar.activation(tmp, tmp, AF.Ln, bias=epsb, scale=1.0)
                nc.scalar.activation(tmp, tmp, AF.Exp, bias=0.0, scale=-0.5)
                nc.vector.tensor_tensor(sc[:, :, 0], tmp, nw, OP.mult)
                nc.vector.tensor_tensor(tmp, gst[:, :, 0], sc[:, :, 0], OP.mult)
                nc.vector.tensor_tensor(sc[:, :, 1], nb, tmp, OP.subtract)
                for ct in range(CT):
                    nc.scalar.activation(h_sb[s][:, ct, :], x_sb[s][:, ct, :], AF.Identity,
                                         bias=sc[:, ct, 1:2], scale=sc[:, ct, 0:1])
                    # x is no longer needed raw; pre-add proj bias for the residual
                    nc.vector.tensor_scalar(x_sb[s][:, ct, :], x_sb[s][:, ct, :],
                                            pb[:, ct:ct + 1], None, OP.add)

            # ---------------- QKV ----------------
            for s in range(S):
                q_sb[s] = qp.tile([128, CT, N], BF16, tag="q", name=f"q{s}")
                k_sb[s] = kp.tile([128, CT, N], BF16, tag="k", name=f"k{s}")
                vT_sb[s] = vp.tile([128, NT, C], BF16, tag="vT", name=f"vT{s}")
                for qk, dst, bias in ((0, q_sb[s], qb), (1, k_sb[s], kb)):
                    for mo in range(CT):
                        ps = ps_big.tile([128, N], F32, tag="big")
                        for nch in range(2):
                            for kc in range(CT):
                                nc.tensor.matmul(
                                    ps[:, nch * 512:(nch + 1) * 512],
                                    lhsT=wT[:, kc, qk * C + mo * 128:qk * C + (mo + 1) * 128],
                                    rhs=h_sb[s][:, kc, nch * 512:(nch + 1) * 512],
                                    start=(kc == 0), stop=(kc == CT - 1))
                        nc.scalar.activation(dst[:, mo, :], ps, AF.Identity,
                                             bias=bias[:, mo:mo + 1], scale=1.0)
                for it in range(NT):
                    ps = ps_mid.tile([128, 512], F32, tag="mid")
                    for kc in range(CT):
                        nc.tensor.matmul(ps, lhsT=h_sb[s][:, kc, it * 128:(it + 1) * 128],
                                         rhs=wT[:, kc, 2 * C:3 * C],
                                         start=(kc == 0), stop=(kc == CT - 1))
                    nc.vector.tensor_copy(vT_sb[s][:, it, :], ps)

            # ---------------- S^T, exp, den ----------------
            for s in range(S):
                es_sb[s] = esp.tile([128, NT, N], BF16, tag="es", name=f"es{s}")
                den_ps = ps_sm.tile([128, NT], F32, tag="den", name=f"den{s}")
                nc.vector.memset(den_ps, 0.0)
                for jt in range(NT):
                    ps = ps_big.tile([128, N], F32, tag="big")
                    for kc in range(CT):
                        for nch in range(2):
                            nc.tensor.matmul(ps[:, nch * 512:(nch + 1) * 512],
                                             lhsT=k_sb[s][:, kc, jt * 128:(jt + 1) * 128],
                                             rhs=q_sb[s][:, kc, nch * 512:(nch + 1) * 512],
                                             start=(kc == 0), stop=(kc == CT - 1))
                    nc.scalar.activation(es_sb[s][:, jt, :], ps, AF.Exp, bias=0.0, scale=SCALE)
                    # den[i] += sum_j(this tile), one tiny matmul per i-chunk; accumulate
                    # into a memset psum bank (start=False: first write per element
                    # overwrites or adds to zeroed data -- correct either way, and these
                    # small matmuls fill PE bubbles during the S^T phase)
                    for ic in range(NT):
                        nc.tensor.matmul(den_ps[:, ic:ic + 1],
                                         lhsT=es_sb[s][:, jt, ic * 128:(ic + 1) * 128],
                                         rhs=ones_bf,
                                         start=False, stop=False, skip_group_check=True)
                recip[s] = statp.tile([128, NT], F32, tag="recip", name=f"recip{s}")
                nc.vector.reciprocal(recip[s], den_ps)

            # ---------------- AV (-> oT[i, c]) ----------------
            for s in range(S):
                oT_sb[s] = otp.tile([128, NT, C], F32, tag="oT", name=f"oT{s}")
                for it in range(NT):
                    ps = ps_mid.tile([128, 512], F32, tag="mid")
                    for jt in range(NT):
                        nc.tensor.matmul(ps, lhsT=es_sb[s][:, jt, it * 128:(it + 1) * 128],
                                         rhs=vT_sb[s][:, jt, :],
                                         start=(jt == 0), stop=(jt == NT - 1))
                    nc.vector.tensor_scalar(oT_sb[s][:, it, :], ps, recip[s][:, it:it + 1],
                                            None, OP.mult)

            # ---------------- transpose oT -> out[c, n] (+bv) ----------------
            for s in range(S):
                ao_sb[s] = aop.tile([128, CT, N], BF16, tag="ao", name=f"ao{s}")
                for ct in range(CT):
                    ps = ps_big.tile([128, N], F32, tag="big")
                    for it in range(NT):
                        nc.tensor.transpose(ps[:, it * 128:(it + 1) * 128],
                                            oT_sb[s][:, it, ct * 128:(ct + 1) * 128], ident)
                    nc.scalar.activation(ao_sb[s][:, ct, :], ps, AF.Identity,
                                         bias=vb[:, ct:ct + 1], scale=1.0)

            # ---------------- proj + residual ----------------
            for s in range(S):
                fin_sb[s] = finp.tile([128, CT, N], F32, tag="fin", name=f"fin{s}")
                for mo in range(CT):
                    for nch in range(2):
                        ps = ps_mid.tile([128, 512], F32, tag="mid")
                        for kc in range(CT):
                            nc.tensor.matmul(ps, lhsT=pwT[:, kc, mo * 128:(mo + 1) * 128],
                                             rhs=ao_sb[s][:, kc, nch * 512:(nch + 1) * 512],
                                             start=(kc == 0), stop=(kc == CT - 1))
                        nc.vector.tensor_tensor(fin_sb[s][:, mo, nch * 512:(nch + 1) * 512],
                                                ps, x_sb[s][:, mo, nch * 512:(nch + 1) * 512],
                                                OP.add)
                for ct in range(CT):
                    nc.sync.dma_start(out_d[s, ct * 128:(ct + 1) * 128, :], fin_sb[s][:, ct, :])

    nc.finalize()
    return nc


_NC_CACHE = None
LAST_EXEC_NS = None
LAST_RESULTS = None


def _get_nc():
    global _NC_CACHE
    if _NC_CACHE is None:
        _NC_CACHE = build_nc()
    return _NC_CACHE


def make_gmat():
    g = np.zeros((128, 128), np.float32)
    g[:64, :64] = 1.0 / 64
    g[64:, 64:] = 1.0 / 64
    return g


def make_in_maps(x, norm_w, norm_b, qkv_w, qkv_b, proj_w, proj_b):
    bf = ml_dtypes.bfloat16
    x = np.asarray(x, np.float32)
    B = x.shape[0]
    x_r = np.ascontiguousarray(x.reshape(B, C, N))
    qkv_wT = np.ascontiguousarray(np.asarray(qkv_w, np.float32).T).astype(bf)
    proj_wT = np.ascontiguousarray(np.asarray(proj_w, np.float32).T).astype(bf)
    common = {
        "qkv_wT": qkv_wT,
        "proj_wT": proj_wT,
        "norm_w": np.ascontiguousarray(np.asarray(norm_w, np.float32)),
        "norm_b": np.ascontiguousarray(np.asarray(norm_b, np.float32)),
        "qkv_b": np.ascontiguousarray(np.asarray(qkv_b, np.float32)),
        "proj_b": np.ascontiguousarray(np.asarray(proj_b, np.float32)),
        "gmat": make_gmat(),
    }
    per = B // NCORES
    return [dict(common, x=np.ascontiguousarray(x_r[c * per:(c + 1) * per]))
            for c in range(NCORES)]


def kernel(x, norm_w, norm_b, qkv_w, qkv_b, proj_w, proj_b, _trace=False):
    global LAST_EXEC_NS, LAST_RESULTS
    x = np.asarray(x)
    B, C_, H, W = x.shape
    in_maps = make_in_maps(x, norm_w, norm_b, qkv_w, qkv_b, proj_w, proj_b)
    res = run_bass_kernel_spmd(_get_nc(), in_maps, core_ids=list(range(NCORES)),
                               trace=_trace)
    LAST_EXEC_NS = res.exec_time_ns
    LAST_RESULTS = res
    out = np.concatenate([res.results[c]["out"] for c in range(NCORES)], axis=0)
    return out.reshape(B, C_, H, W).astype(np.float32)


# revision 13
# speedup vs baseline: 1.3440x; 1.1791x over previous
"""AttentionBlock (GroupNorm + single-head self-attention + proj + residual) on 8 Trainium2
NeuronCores, data-parallel over the batch (16 samples -> 2 per core).

Per-sample math (C=512 channels, N=1024 tokens = 32x32 spatial):
  h   = GroupNorm(x; 8 groups) * w + b           [C, N]
  q,k = Wq@h + bq, Wk@h + bk                     [C, N]   (c on partitions)
  vT  = h^T @ WvT (+bv deferred past softmax)    [N, C]   (tokens on partitions)
  S^T = k^T q * scale                            [j, i]   (j on partitions)
  E   = exp(S^T)       den[i] = sum_j E[j,i]     (den via ones-matmuls, i on partitions)
  oT[i,c] = sum_j E[j,i] vT[j,c];  oT *= 1/den[i]
  out = transpose(oT) + bv                       [C, N]
  y   = (x + bp) + Wp@out                        [C, N]

All matmul operands are bf16 (PSUM accumulates fp32); softmax/statistics are fp32.
Softmax skips max-subtraction: scores are ~N(0,1) (|s| < ~7), exp() is safe in fp32.
"""

import numpy as np
import ml_dtypes

import concourse.bacc as bacc
import concourse.tile as tile
from concourse import mybir
from concourse.bass_utils import run_bass_kernel_spmd
from concourse.hw_specs import get_activation_tables as _gat
from concourse.masks import make_identity

F32 = mybir.dt.float32
BF16 = mybir.dt.bfloat16
AF = mybir.ActivationFunctionType
OP = mybir.AluOpType

NCORES = 8
S = 2          # samples per core
C = 512
N = 1024       # H*W
CT = C // 128  # channel tiles
NT = N // 128  # token tiles
EPS = 1e-5
SCALE = float(C) ** -0.5

# All ACT funcs we use (Exp, Ln, Identity, Copy) live in one table set; blank out the
# other sets (keeping list positions!) so the table-load pass never alternates sets.
_ONE_SET = "natural_log_exp_and_others"


def _gat_filtered(arch):
    return {name: (fns if name == _ONE_SET else set())
            for name, fns in _gat(arch).items()}


bacc.get_activation_tables = _gat_filtered


def build_nc():
    nc = bacc.Bacc("TRN2", target_bir_lowering=False)
    x_d = nc.dram_tensor("x", [S, C, N], F32, kind="ExternalInput")
    wT_d = nc.dram_tensor("qkv_wT", [C, 3 * C], BF16, kind="ExternalInput")
    pwT_d = nc.dram_tensor("proj_wT", [C, C], BF16, kind="ExternalInput")
    nw_d = nc.dram_tensor("norm_w", [C], F32, kind="ExternalInput")
    nb_d = nc.dram_tensor("norm_b", [C], F32, kind="ExternalInput")
    qkvb_d = nc.dram_tensor("qkv_b", [3 * C], F32, kind="ExternalInput")
    pb_d = nc.dram_tensor("proj_b", [C], F32, kind="ExternalInput")
    gm_d = nc.dram_tensor("gmat", [128, 128], F32, kind="ExternalInput")
    out_d = nc.dram_tensor("out", [S, C, N], F32, kind="ExternalOutput")

    with tile.TileContext(nc) as tc:
        with (
            tc.tile_pool(name="consts", bufs=1) as consts,
            tc.tile_pool(name="xp", bufs=2) as xp,
            tc.tile_pool(name="hp", bufs=2) as hp,
            tc.tile_pool(name="qp", bufs=2) as qp,
            tc.tile_pool(name="kp", bufs=2) as kp,
            tc.tile_pool(name="vp", bufs=2) as vp,
            tc.tile_pool(name="esp", bufs=1) as esp,
            tc.tile_pool(name="otp", bufs=1) as otp,
            tc.tile_pool(name="aop", bufs=1) as aop,
            tc.tile_pool(name="finp", bufs=2) as finp,
            tc.tile_pool(name="statp", bufs=4) as statp,
            tc.tile_pool(name="ps_big", bufs=2, space="PSUM") as ps_big,
            tc.tile_pool(name="ps_mid", bufs=2, space="PSUM") as ps_mid,
            tc.tile_pool(name="ps_sm", bufs=1, space="PSUM") as ps_sm,
        ):
            # ---------------- load x(s0) and weights first, then x(s1) ----------
            x_sb, h_sb, q_sb, k_sb, vT_sb = {}, {}, {}, {}, {}
            es_sb, oT_sb, ao_sb, fin_sb, recip = {}, {}, {}, {}, {}
            wT = consts.tile([128, CT, 3 * C], BF16, tag="wT")
            wT_r = wT_d.ap().rearrange("(kc p) o -> p kc o", p=128)
            for s in range(S):
                x_sb[s] = xp.tile([128, CT, N], F32, tag="x", name=f"x{s}")
            for ct in range(CT):
                for hh in range(2):
                    nc.sync.dma_start(x_sb[0][:, ct, hh * 512:(hh + 1) * 512],
                                      x_d[0, ct * 128:(ct + 1) * 128, hh * 512:(hh + 1) * 512])
            for kc in range(CT):
                nc.sync.dma_start(wT[:, kc, :], wT_r[:, kc, :])
            for ct in range(CT):
                nc.sync.dma_start(x_sb[1][:, ct, :], x_d[1, ct * 128:(ct + 1) * 128, :])

            # ---------------- constants ----------------
            pwT = consts.tile([128, CT, C], BF16, tag="pwT")
            pwT_r = pwT_d.ap().rearrange("(kc p) o -> p kc o", p=128)
            for kc in range(CT):
                nc.sync.dma_start(pwT[:, kc, :], pwT_r[:, kc, :])
            gmat = consts.tile([128, 128], F32, tag="gmat")
            nc.sync.dma_start(gmat, gm_d.ap())
            ident = consts.tile([128, 128], F32, tag="ident")
            make_identity(nc, ident)
            ones_bf = consts.tile([128, 1], BF16, tag="ones")
            nc.vector.memset(ones_bf, 1.0)
            epsb = consts.tile([128, 1], F32, tag="eps")
            nc.vector.memset(epsb, EPS)
            nw = consts.tile([128, CT], F32, tag="nw")
            nc.sync.dma_start(nw, nw_d.ap().rearrange("(t p) -> p t", p=128))
            nb = consts.tile([128, CT], F32, tag="nb")
            nc.sync.dma_start(nb, nb_d.ap().rearrange("(t p) -> p t", p=128))
            qb = consts.tile([128, CT], F32, tag="qb")
            nc.sync.dma_start(qb, qkvb_d.ap()[0:C].rearrange("(t p) -> p t", p=128))
            kb = consts.tile([128, CT], F32, tag="kb")
            nc.sync.dma_start(kb, qkvb_d.ap()[C:2 * C].rearrange("(t p) -> p t", p=128))
            vb = consts.tile([128, CT], F32, tag="vb")
            nc.sync.dma_start(vb, qkvb_d.ap()[2 * C:3 * C].rearrange("(t p) -> p t", p=128))
            pb = consts.tile([128, CT], F32, tag="pb")
            nc.sync.dma_start(pb, pb_d.ap().rearrange("(t p) -> p t", p=128))

            # warm the ACT table set under the DMA shadow
            warm = statp.tile([128, 1], F32, tag="tmp", name="warm")
            nc.scalar.activation(warm, epsb, AF.Exp, bias=0.0, scale=1.0)
            # warm-up matmuls: keep the PE busy (and the HAM clock-gate open) while
            # the x DMAs and groupnorm statistics run; results are never read
            for wi in range(24):
                wps = ps_big.tile([128, N], F32, tag="big", name=f"warm{wi}")
                nc.tensor.matmul(wps[:, 0:128], lhsT=ident, rhs=ident, start=True, stop=True)


            for s in range(S):
                h_sb[s] = hp.tile([128, CT, N], BF16, tag="h", name=f"h{s}")
                # per-partition (mean, E[x^2]) for all 4 c-tiles: mv[:, ct, 0:2]
                mv = statp.tile([128, CT, 2], F32, tag="mv", name=f"mv{s}")
                for ct in range(CT):
                    st = statp.tile([128, 2, 6], F32, tag="bnst")
                    for i in range(2):
                        nc.vector.bn_stats(st[:, i, :], x_sb[s][:, ct, i * 512:(i + 1) * 512])
                    nc.vector.bn_aggr(mv[:, ct, :], st)
                # E[x^2] = var + mean^2 (batched over all c-tiles, strided [128,4] views)
                msq = statp.tile([128, CT, 2], F32, tag="msq", name=f"msq{s}")
                nc.vector.tensor_copy(msq[:, :, 0], mv[:, :, 0])
                nc.vector.tensor_tensor(msq[:, :, 1], mv[:, :, 0], mv[:, :, 0], OP.mult)
                nc.vector.tensor_tensor(msq[:, :, 1], msq[:, :, 1], mv[:, :, 1], OP.add)
                # group-average + broadcast back to all partitions: ONE matmul, all c-tiles
                gps = ps_sm.tile([128, 2 * CT], F32, tag="gnagg", name=f"gps{s}")
                nc.tensor.matmul(gps[:, 0:2 * CT], lhsT=gmat,
                                 rhs=msq.rearrange("p a b -> p (a b)"),
                                 start=True, stop=True)
                gst = statp.tile([128, CT, 2], F32, tag="gst", name=f"gst{s}")
                nc.vector.tensor_copy(gst.rearrange("p a b -> p (a b)"), gps[:, 0:2 * CT])
                # scale = rstd * w ; shift = b - mean * scale   (all c-tiles at once)
                sc = statp.tile([128, CT, 2], F32, tag="sc", name=f"sc{s}")
                tmp = statp.tile([128, CT], F32, tag="tmp", name=f"tmp{s}")
                nc.vector.tensor_tensor(tmp, gst[:, :, 0], gst[:, :, 0], OP.mult)
                nc.vector.tensor_tensor(tmp, gst[:, :, 1], tmp, OP.subtract)  # var
                # rstd = exp(-0.5*ln(var+eps)); Ln+Exp live in one ACT table set
                nc.scalar.activation(tmp, tmp, AF.Ln, bias=epsb, scale=1.0)
                nc.scalar.activation(tmp, tmp, AF.Exp, bias=0.0, scale=-0.5)
                nc.vector.tensor_tensor(sc[:, :, 0], tmp, nw, OP.mult)
                nc.vector.tensor_tensor(tmp, gst[:, :, 0], sc[:, :, 0], OP.mult)
                nc.vector.tensor_tensor(sc[:, :, 1], nb, tmp, OP.subtract)
                for ct in range(CT):
                    nc.scalar.activation(h_sb[s][:, ct, :], x_sb[s][:, ct, :], AF.Identity,
                                         bias=sc[:, ct, 1:2], scale=sc[:, ct, 0:1])
                    # x is no longer needed raw; pre-add proj bias for the residual
                    nc.vector.tensor_scalar(x_sb[s][:, ct, :], x_sb[s][:, ct, :],
                                            pb[:, ct:ct + 1], None, OP.add)

            # ---------------- QKV ----------------
            for s in range(S):
                q_sb[s] = qp.tile([128, CT, N], BF16, tag="q", name=f"q{s}")
                k_sb[s] = kp.tile([128, CT, N], BF16, tag="k", name=f"k{s}")
                vT_sb[s] = vp.tile([128, NT, C], BF16, tag="vT", name=f"vT{s}")
                for qk, dst, bias in ((0, q_sb[s], qb), (1, k_sb[s], kb)):
                    for mo in range(CT):
                        ps = ps_big.tile([128, N], F32, tag="big")
                        for nch in range(2):
                            for kc in range(CT):
                                nc.tensor.matmul(
                                    ps[:, nch * 512:(nch + 1) * 512],
                                    lhsT=wT[:, kc, qk * C + mo * 128:qk * C + (mo + 1) * 128],
                                    rhs=h_sb[s][:, kc, nch * 512:(nch + 1) * 512],
                                    start=(kc == 0), stop=(kc == CT - 1))
                        nc.scalar.activation(dst[:, mo, :], ps, AF.Identity,
                                             bias=bias[:, mo:mo + 1], scale=1.0)
                for it in range(NT):
                    ps = ps_mid.tile([128, 512], F32, tag="mid")
                    for kc in range(CT):
                        nc.tensor.matmul(ps, lhsT=h_sb[s][:, kc, it * 128:(it + 1) * 128],
                                         rhs=wT[:, kc, 2 * C:3 * C],
                                         start=(kc == 0), stop=(kc == CT - 1))
                    nc.vector.tensor_copy(vT_sb[s][:, it, :], ps)

            # ---------------- S^T, exp, den ----------------
            for s in range(S):
                es_sb[s] = esp.tile([128, NT, N], BF16, tag="es", name=f"es{s}")
                den_ps = ps_sm.tile([128, NT], F32, tag="den", name=f"den{s}")
                nc.vector.memset(den_ps, 0.0)
                for jt in range(NT):
                    ps = ps_big.tile([128, N], F32, tag="big")
                    for kc in range(CT):
                        for nch in range(2):
                            nc.tensor.matmul(ps[:, nch * 512:(nch + 1) * 512],
                                             lhsT=k_sb[s][:, kc, jt * 128:(jt + 1) * 128],
                                             rhs=q_sb[s][:, kc, nch * 512:(nch + 1) * 512],
                                             start=(kc == 0), stop=(kc == CT - 1))
                    nc.scalar.activation(es_sb[s][:, jt, :], ps, AF.Exp, bias=0.0, scale=SCALE)
                    # den[i] += sum_j(this tile), one tiny matmul per i-chunk; accumulate
                    # into a memset psum bank (start=False: first write per element
                    # overwrites or adds to zeroed data -- correct either way, and these
                    # small matmuls fill PE bubbles during the S^T phase)
                    for ic in range(NT):
                        nc.tensor.matmul(den_ps[:, ic:ic + 1],
                                         lhsT=es_sb[s][:, jt, ic * 128:(ic + 1) * 128],
                                         rhs=ones_bf,
                                         start=False, stop=False, skip_group_check=True)
                recip[s] = statp.tile([128, NT], F32, tag="recip", name=f"recip{s}")
                nc.vector.reciprocal(recip[s], den_ps)

            # ---------------- AV (-> oT[i, c]) ----------------
            for s in range(S):
                oT_sb[s] = otp.tile([128, NT, C], F32, tag="oT", name=f"oT{s}")
                for it in range(NT):
                    ps = ps_mid.tile([128, 512], F32, tag="mid")
                    for jt in range(NT):
                        nc.tensor.matmul(ps, lhsT=es_sb[s][:, jt, it * 128:(it + 1) * 128],
                                         rhs=vT_sb[s][:, jt, :],
                                         start=(jt == 0), stop=(jt == NT - 1))
                    nc.vector.tensor_scalar(oT_sb[s][:, it, :], ps, recip[s][:, it:it + 1],
                                            None, OP.mult)

            # ---------------- transpose oT -> out[c, n] (+bv) ----------------
            for s in range(S):
                ao_sb[s] = aop.tile([128, CT, N], BF16, tag="ao", name=f"ao{s}")
                for ct in range(CT):
                    ps = ps_big.tile([128, N], F32, tag="big")
                    for it in range(NT):
                        nc.tensor.transpose(ps[:, it * 128:(it + 1) * 128],
                                            oT_sb[s][:, it, ct * 128:(ct + 1) * 128], ident)
                    nc.scalar.activation(ao_sb[s][:, ct, :], ps, AF.Identity,
                                         bias=vb[:, ct:ct + 1], scale=1.0)

            # ---------------- proj + residual ----------------
            for s in range(S):
                fin_sb[s] = finp.tile([128, CT, N], F32, tag="fin", name=f"fin{s}")
                for mo in range(CT):
                    for nch in range(2):
                        ps = ps_mid.tile([128, 512], F32, tag="mid")
                        for kc in range(CT):
                            nc.tensor.matmul(ps, lhsT=pwT[:, kc, mo * 128:(mo + 1) * 128],
                                             rhs=ao_sb[s][:, kc, nch * 512:(nch + 1) * 512],
                                             start=(kc == 0), stop=(kc == CT - 1))
                        nc.vector.tensor_tensor(fin_sb[s][:, mo, nch * 512:(nch + 1) * 512],
                                                ps, x_sb[s][:, mo, nch * 512:(nch + 1) * 512],
                                                OP.add)
                for ct in range(CT):
                    nc.sync.dma_start(out_d[s, ct * 128:(ct + 1) * 128, :], fin_sb[s][:, ct, :])

    nc.finalize()
    return nc


_NC_CACHE = None
LAST_EXEC_NS = None
LAST_RESULTS = None


def _get_nc():
    global _NC_CACHE
    if _NC_CACHE is None:
        _NC_CACHE = build_nc()
    return _NC_CACHE


def make_gmat():
    g = np.zeros((128, 128), np.float32)
    g[:64, :64] = 1.0 / 64
    g[64:, 64:] = 1.0 / 64
    return g


def make_in_maps(x, norm_w, norm_b, qkv_w, qkv_b, proj_w, proj_b):
    bf = ml_dtypes.bfloat16
    x = np.asarray(x, np.float32)
    B = x.shape[0]
    x_r = np.ascontiguousarray(x.reshape(B, C, N))
    qkv_wT = np.ascontiguousarray(np.asarray(qkv_w, np.float32).T).astype(bf)
    proj_wT = np.ascontiguousarray(np.asarray(proj_w, np.float32).T).astype(bf)
    common = {
        "qkv_wT": qkv_wT,
        "proj_wT": proj_wT,
        "norm_w": np.ascontiguousarray(np.asarray(norm_w, np.float32)),
        "norm_b": np.ascontiguousarray(np.asarray(norm_b, np.float32)),
        "qkv_b": np.ascontiguousarray(np.asarray(qkv_b, np.float32)),
        "proj_b": np.ascontiguousarray(np.asarray(proj_b, np.float32)),
        "gmat": make_gmat(),
    }
    per = B // NCORES
    return [dict(common, x=np.ascontiguousarray(x_r[c * per:(c + 1) * per]))
            for c in range(NCORES)]


def kernel(x, norm_w, norm_b, qkv_w, qkv_b, proj_w, proj_b, _trace=False):
    global LAST_EXEC_NS, LAST_RESULTS
    x = np.asarray(x)
    B, C_, H, W = x.shape
    in_maps = make_in_maps(x, norm_w, norm_b, qkv_w, qkv_b, proj_w, proj_b)
    res = run_bass_kernel_spmd(_get_nc(), in_maps, core_ids=list(range(NCORES)),
                               trace=_trace)
    LAST_EXEC_NS = res.exec_time_ns
    LAST_RESULTS = res
    out = np.concatenate([res.results[c]["out"] for c in range(NCORES)], axis=0)
    return out.reshape(B, C_, H, W).astype(np.float32)
